# revision 23
# baseline (speedup 1.0000x reference)
"""Trainium2 Bass kernel: GNN message passing (nn_BaseAC_22505628631094).

Computation (see reference):
  transformed = feature_src @ fc_W.T + fc_b                     [N_src, TFD]
  h_src/h_dst = per-head emb @ att_W                            [H, N, HID]
  scores      = elu((h_dst @ att_W2) @ h_src.T)                 [H, N_dest, N_src]
  attn        = softmax(where(bias>0, scores, -inf), axis=-1)
  out_re      = mean_h(attn @ transformed)                      [N_dest, TFD]
  out_hat     = transformed @ dec_W.T + dec_b                   [N_src, FEAT]

Sharding: N_dest rows split across 8 cores (attention / softmax rows are
independent); feature_hat is sharded by N_src rows.  Everything is computed
in a transposed "scoresT" layout [src, dest] so that the attention matrix
comes out of the softmax directly usable as matmul lhsT for aggregation —
no on-device transposes of the big matrices.

Softmax numerator exp(elu(x))*mask is computed exactly with stock ops:
    u = exp(x)                       (ScalarE)
    z = min(u, M1), M1 in {1, -448}  (VectorE; clamp + mask in one op)
    v = exp(z - 1)                   (ScalarE; masked -> exp(-449) == 0)
    e = max(u, 1) * v                (VectorE scalar_tensor_tensor)
For x>0: v = exp(0) = 1, e = u = exp(x) exactly; for x<=0: e = exp(u-1) =
exp(elu(x)) exactly; masked entries are exactly 0.
Row sums (softmax denominators) fall out of the aggregation matmul via an
extra ones-column appended to `transformed`.
"""

import sys

import numpy as np

sys.path.insert(0, "/opt/trn_rl_repo")

import ml_dtypes  # noqa: E402

import concourse.bass as bass  # noqa: E402
import concourse.mybir as mybir  # noqa: E402
import concourse.tile as tile  # noqa: E402
from contextlib import ExitStack  # noqa: E402

# ---------------- problem constants (hardcoded per harness contract) -------
N_DEST = 8192
N_SRC = 8192
FEAT, TFD, EMB, HID, HEADS = 1024, 256, 64, 128, 2
NCORES = 8
M_PER = N_DEST // NCORES          # 1024 dest rows per core
S_TILES = N_SRC // 128            # 64 src tiles
CHUNK = 512                       # dest columns per (head, chunk) pass
N_CHUNKS = M_PER // CHUNK         # 2
DT_PER = CHUNK // 128             # 4 dest sub-tiles per chunk
KF = FEAT // 128 + 1              # 8 feature K-tiles + 1 bias-row tile
FS_PAD = KF * 128                 # 1152
OWN_TILES = S_TILES // NCORES     # 8 src tiles owned per core (feature_hat)
TA_W = 258                        # transformed_aug width: TFD + ones-col + pad
                                  # (col 256 = softmax-denominator ones column;
                                  # col 257 = zero pad: fp32r matmul needs even N)

BIAS_DT = mybir.dt.float8e4
BIAS_NP = ml_dtypes.float8_e4m3

# q(u) = Q0 + u*(Q1 + Q2*u) ~ exp((u-1)/2) on [0, 1]  (minimax fit, so that
# min(q,1)^2 ~ exp(u-1) with ~1.3e-3 max rel err; exact branch for u>1)
Q0, Q1, Q2 = 0.60692452, 0.29545556, 0.09697007

F32 = mybir.dt.float32
F32R = mybir.dt.float32r
BF16 = mybir.dt.bfloat16
EXP = mybir.ActivationFunctionType.Exp
MULT = mybir.AluOpType.mult


# ---------------- custom DVE op: fused exp(elu(.)) * mask ------------------
_EXP_ELU_OP = None


def _register_exp_elu_mask():
    global _EXP_ELU_OP
    if _EXP_ELU_OP is not None:
        return _EXP_ELU_OP
    import concourse.dve_ops as dvo
    from concourse.dve_spec import (
        C0, C1, C2, One, Spec, Src0, Src1, lower, maxx, minn, sq,
    )
    from concourse.dve_uop import DveOpSpec

    name = "EXP_ELU_MASK_ANT"
    if name in dvo._SUB_OPCODE_FOR_NAME:
        _EXP_ELU_OP = next(op for op in dvo.OPS if op.name == name)
        return _EXP_ELU_OP

    def _ref(in0, in1, s0, s1, imm2):
        u = np.asarray(in0, np.float32)
        q = np.float32(s0) + u * (np.float32(s1) + np.float32(imm2) * u)
        p = np.minimum(q, np.float32(1.0)) ** 2
        g = np.maximum(u, p)
        return (g * np.asarray(in1, np.float32)).astype(np.float32)

    body = maxx(Src0, sq(minn(C0 + Src0 * (C1 + C2 * Src0), One))) * Src1
    spec = Spec(body=body, reference=_ref)
    row = max(dvo._SUB_OPCODE_FOR_NAME.values()) + 1
    assert row < 0x20, "custom DVE opcode rows exhausted"
    shas = {}
    for ver in ("v3", "v4"):
        s = DveOpSpec(name=name, opcode=row, uops=lower(spec, ver=ver),
                      rd1_en=True)
        shas[ver] = s.sha(ver)
    op = dvo.DveOp(name, spec, subdim=False, uops_sha=shas)
    dvo.OPS.append(op)
    dvo.CUSTOM_DVE_SPECS[name] = spec
    dvo._SUB_OPCODE_FOR_NAME[name] = row
    _EXP_ELU_OP = op
    return op


# ---------------- device program -------------------------------------------
def _kernel_body(tc, I, O):
    nc = tc.nc

    with ExitStack() as ctx:
        # ---- whole-kernel SBUF pools ----
        consts = ctx.enter_context(tc.tile_pool(name="consts", bufs=1))
        big = ctx.enter_context(tc.tile_pool(name="big", bufs=1))
        tfa_pool = ctx.enter_context(tc.tile_pool(name="tfa", bufs=S_TILES))

        # constants
        fcWT = consts.tile([128, KF * TA_W], F32R, name="fcWT")
        for k in range(KF):
            nc.sync.dma_start(fcWT[:, k * TA_W:(k + 1) * TA_W], I["fcWT"][k])
        attW = consts.tile([EMB, HEADS * HID], F32R, name="attW")
        for h in range(HEADS):
            nc.sync.dma_start(attW[:, h * HID:(h + 1) * HID], I["attW"][h])
        attW2 = consts.tile([HID, HEADS * HID], F32R, name="attW2")
        for h in range(HEADS):
            nc.sync.dma_start(attW2[:, h * HID:(h + 1) * HID], I["attW2"][h])
        embdT = consts.tile([EMB, M_PER], F32R, name="embdT")
        nc.sync.dma_start(embdT[:], I["embdT"][:])
        ident = consts.tile([128, 128], F32, name="ident")
        nc.sync.dma_start(ident[:], I["ident"][:])
        onesr = consts.tile([1, 128], F32R, name="onesr")
        nc.sync.dma_start(onesr[:], I["onesr"][:])
        decA = consts.tile([128, FEAT], F32R, name="decA")
        nc.sync.dma_start(decA[:], I["decWT"][0:128, :])
        decB = consts.tile([128, FEAT], F32R, name="decB")
        nc.sync.dma_start(decB[:], I["decWT"][128:256, :])
        decC = consts.tile([1, FEAT], F32R, name="decC")
        nc.sync.dma_start(decC[:], I["decWT"][256:257, :])
        neg1 = consts.tile([128, 1], F32, name="neg1")
        nc.gpsimd.memset(neg1[:], -1.0)

        # big persistent tiles
        hsrc = big.tile([HID, HEADS * N_SRC], F32R, name="hsrc")   # 64KB/part
        hd2 = big.tile([HID, HEADS * M_PER], F32R, name="hd2")
        hdT = big.tile([HID, M_PER], F32R, name="hdT")
        oacc = big.tile([128, 8 * TFD], F32, name="oacc")
        own_tf = big.tile([128, OWN_TILES * TA_W], F32, name="own_tf")

        # ---- Phase A: h_srcT / hd2T for both heads --------------------------
        with tc.tile_pool(name="embs_p", bufs=1) as embs_p, \
             tc.tile_pool(name="ps_hs", bufs=2, space="PSUM") as ps_hs, \
             tc.tile_pool(name="ps_hd", bufs=1, space="PSUM") as ps_hd:
            embsT = embs_p.tile([EMB, N_SRC], F32R, name="embsT")
            nc.sync.dma_start(embsT[:], I["embsT"][:])
            for h in range(HEADS):
                aW = attW[:, h * HID:(h + 1) * HID]
                aW2 = attW2[:, h * HID:(h + 1) * HID]
                # h_srcT[h] = (emb_src @ att_W[h]).T = att_W[h].T @ emb_src.T
                for n in range(N_SRC // 512):
                    ph = ps_hs.tile([128, 512], F32, tag="hs", name="ph")
                    nc.tensor.matmul(ph[:], aW, embsT[:, n * 512:(n + 1) * 512],
                                     start=True, stop=True)
                    nc.vector.tensor_copy(hsrc[:, h * N_SRC + n * 512: h * N_SRC + (n + 1) * 512],
                                          ph[:])
                # hdT = att_W[h].T @ emb_destT ; hd2T = att_W2[h].T @ hdT
                pd = ps_hd.tile([128, M_PER], F32, tag="hd", name="pd")
                for n in range(M_PER // 512):
                    nc.tensor.matmul(pd[:, n * 512:(n + 1) * 512], aW,
                                     embdT[:, n * 512:(n + 1) * 512],
                                     start=True, stop=True)
                nc.vector.tensor_copy(hdT[:], pd[:])
                pd2 = ps_hd.tile([128, M_PER], F32, tag="hd", name="pd2")
                for n in range(M_PER // 512):
                    nc.tensor.matmul(pd2[:, n * 512:(n + 1) * 512], aW2,
                                     hdT[:, n * 512:(n + 1) * 512],
                                     start=True, stop=True)
                nc.vector.tensor_copy(hd2[:, h * M_PER + 0: h * M_PER + M_PER], pd2[:])

        # ---- Phases B-E: transformed (streamed) + attention main loop -------
        tfa = []  # 64 bf16 tiles [128, 257]: transformed_aug, ones in col 256

        # ---- transformed pre-phase (streams feature_src; overlaps phase A) --
        with tc.tile_pool(name="ps_tf", bufs=2, space="PSUM") as ps_tf, \
             tc.tile_pool(name="slab_p", bufs=3) as slab_p:
            for st in range(S_TILES):
                slab = slab_p.tile([128, FS_PAD], F32R, tag="slab", name="slab")
                nc.sync.dma_start(slab[:], I["fsT9"][st])
                pt = ps_tf.tile([128, TA_W], F32, tag="ptf", name="pt")
                for k in range(KF):
                    nc.tensor.matmul(pt[:], slab[:, k * 128:(k + 1) * 128],
                                     fcWT[:, k * TA_W:(k + 1) * TA_W],
                                     start=(k == 0), stop=(k == KF - 1))
                t = tfa_pool.tile([128, TA_W], BF16, tag="tfa", name="tfa_t")
                nc.vector.tensor_copy(t[:], pt[:])
                if st < OWN_TILES:
                    nc.scalar.copy(own_tf[:, st * TA_W:(st + 1) * TA_W], pt[:])
                tfa.append(t)

        # ---- main attention loop: head x chunk x src-tile-pair --------------
        with tc.tile_pool(name="ps_sc", bufs=2, space="PSUM") as ps_sc, \
             tc.tile_pool(name="mask_p", bufs=4) as mask_p, \
             tc.tile_pool(name="ps_agg", bufs=4, space="PSUM") as ps_agg, \
             tc.tile_pool(name="u_p", bufs=3) as u_p, \
             tc.tile_pool(name="z_p", bufs=3) as z_p, \
             tc.tile_pool(name="v_p", bufs=3) as v_p, \
             tc.tile_pool(name="e_p", bufs=3) as e_p, \
             tc.tile_pool(name="nrm_p", bufs=2) as nrm_p:

            def main_iter(h, c, pr, agg):
                # two src tiles per iteration share one [128, 1024] pipeline
                m1 = mask_p.tile([128, 2 * CHUNK], BF16, tag="m1", name="m1")
                nc.sync.dma_start(m1[:], I["biasM1"][c, pr])
                ps = ps_sc.tile([128, 2 * CHUNK], F32, tag="sc", name="ps")
                for half in range(2):
                    st = 2 * pr + half
                    nc.tensor.matmul(
                        ps[:, half * CHUNK:(half + 1) * CHUNK],
                        hsrc[:, h * N_SRC + st * 128: h * N_SRC + (st + 1) * 128],
                        hd2[:, h * M_PER + c * CHUNK: h * M_PER + (c + 1) * CHUNK],
                        start=True, stop=True)
                u = u_p.tile([128, 2 * CHUNK], BF16, tag="u", name="u")
                nc.scalar.activation(u[:], ps[:], EXP)
                z = z_p.tile([128, 2 * CHUNK], BF16, tag="z", name="z")
                nc.vector.tensor_tensor(z[:], u[:], m1[:], mybir.AluOpType.min)
                v = v_p.tile([128, 2 * CHUNK], BF16, tag="v", name="v")
                nc.scalar.activation(v[:], z[:], EXP, bias=neg1[:])
                e = e_p.tile([128, 2 * CHUNK], BF16, tag="e", name="e")
                nc.vector.scalar_tensor_tensor(e[:], u[:], 1.0, v[:],
                                               mybir.AluOpType.max, MULT)
                for half in range(2):
                    st = 2 * pr + half
                    for dt in range(DT_PER):
                        nc.tensor.matmul(
                            agg[dt][:],
                            e[:, half * CHUNK + dt * 128: half * CHUNK + (dt + 1) * 128],
                            tfa[st][:], start=(st == 0),
                            stop=(st == S_TILES - 1))

            def normalize(h, c, agg):
                for dt in range(DT_PER):
                    rden = nrm_p.tile([128, 1], F32, tag="rden", name="rden")
                    nc.vector.reciprocal(rden[:], agg[dt][:, 256:257])
                    m = c * DT_PER + dt
                    dst = oacc[:, m * TFD:(m + 1) * TFD]
                    if h == 0:
                        nc.vector.tensor_scalar(dst, agg[dt][:, 0:256], rden[:],
                                                0.5, MULT, MULT)
                    else:
                        tmp = nrm_p.tile([128, TFD], F32, tag="ntmp", name="ntmp")
                        nc.vector.tensor_scalar(tmp[:], agg[dt][:, 0:256], rden[:],
                                                0.5, MULT, MULT)
                        nc.vector.tensor_add(dst, dst, tmp[:])

            for h in range(HEADS):
                for c in range(N_CHUNKS):
                    agg = [ps_agg.tile([128, TA_W], F32, tag="agg", name=f"agg{dt}")
                           for dt in range(DT_PER)]
                    for pr in range(S_TILES // 2):
                        main_iter(h, c, pr, agg)
                    normalize(h, c, agg)

        # out_re DMA
        for m in range(8):
            nc.sync.dma_start(O["out_re"][m * 128:(m + 1) * 128, :],
                              oacc[:, m * TFD:(m + 1) * TFD])

        # ---- Phase F: feature_hat for this core's own 8 src tiles -----------
        with tc.tile_pool(name="ps_tr", bufs=2, space="PSUM") as ps_tr, \
             tc.tile_pool(name="ps_fh", bufs=2, space="PSUM") as ps_fh, \
             tc.tile_pool(name="hat_p", bufs=2) as hat_p:
            for j in range(OWN_TILES):
                tt = hat_p.tile([128, 256], F32R, tag="tt", name="tt")
                for kt in range(2):
                    ptr = ps_tr.tile([128, 128], F32, tag="tr", name="ptr")
                    nc.tensor.transpose(
                        ptr[:], own_tf[:, j * TA_W + kt * 128: j * TA_W + (kt + 1) * 128],
                        ident[:])
                    nc.scalar.copy(tt[:, kt * 128:(kt + 1) * 128], ptr[:])
                pfh = ps_fh.tile([128, FEAT], F32, tag="fh", name="pfh")
                for nf in range(2):
                    o = pfh[:, nf * 512:(nf + 1) * 512]
                    for kt in range(2):
                        nc.tensor.matmul(o, tt[:, kt * 128:(kt + 1) * 128],
                                         (decA if kt == 0 else decB)[:, nf * 512:(nf + 1) * 512],
                                         start=(kt == 0), stop=False)
                    nc.tensor.matmul(o, onesr[:], decC[:, nf * 512:(nf + 1) * 512],
                                     start=False, stop=True)
                fh = hat_p.tile([128, FEAT], F32, tag="fh_sb", name="fh")
                nc.vector.tensor_copy(fh[:], pfh[:])
                nc.sync.dma_start(O["out_hat"][j * 128:(j + 1) * 128, :], fh[:])


def _cap_pe_waits(nc):
    """Walrus codegen allows only one embedded sync-wait per compute-engine
    instruction (PE Matmult, ACT Activation, DVE ops, ...).  Tile's semaphore
    assignment can attach several; split the excess onto same-engine no-ops
    inserted immediately before — identical semantics, ~free."""
    import bass_rust
    k = 0
    # dummy semaphore for wait-carrier EVSEMs on SP (EVSEM needs an update)
    sem_names = dict(nc.m.ant_sem_names)
    dummy_id = max(int(i) for i in sem_names) + 1
    sem_names[str(dummy_id)] = ["wnop_dummy"]
    nc.m.ant_sem_names = sem_names
    dummy_upd = bass_rust.SyncUpdate(
        sync_type="semaphore", id=dummy_id, ant_name="wnop_dummy",
        update_mode="sem-inc", update_value=1, update_reg=None)
    skip = ("InstNoOp", "InstEventSemaphore",
            "InstAllEngineBarrier", "InstUnconditionalBranch", "InstISA",
            "InstBranchHint")
    cap_engines = {mybir.EngineType.PE, mybir.EngineType.DVE,
                   mybir.EngineType.Activation, mybir.EngineType.Pool,
                   mybir.EngineType.SP}
    for f in nc.m.functions:
        for blk in f.blocks:
            insts = blk.instructions
            out = []
            changed = False
            for inst in insts:
                if (type(inst).__name__ == "InstISA"
                        and getattr(inst, "op_name", None)
                        == "EVENT_SEMAPHORE_RANGE_CLEAR"):
                    # this walrus build rejects the encoding ("ISA wrong
                    # length"); the preceding reset-sema Drain already zeroes
                    # the semaphore range, so drop it
                    changed = True
                    continue
                si = inst.sync_info
                if (si is not None and type(inst).__name__ not in skip
                        and inst.engine in cap_engines):
                    waits = list(si.on_wait)
                    if len(waits) > 1:
                        for w in waits[:-1]:
                            if inst.engine == mybir.EngineType.SP:
                                nop = bass_rust.InstEventSemaphore(
                                    name=f"I-wnop{k}", ins=[], outs=[])
                                upd = [dummy_upd]
                            else:
                                nop = bass_rust.InstNoOp(
                                    name=f"I-wnop{k}", ins=[], outs=[])
                                upd = []
                            k += 1
                            nop.engine = inst.engine
                            nop.sync_info = bass_rust.SyncInfo(
                                on_wait=[w], on_update=upd)
                            out.append(nop)
                        si.on_wait = waits[-1:]
                        changed = True
                out.append(inst)
            if changed:
                blk.instructions = out
    return k


_PROGRAM = None


def _build_program():
    nc = bass.Bass("TRN2", target_bir_lowering=False, debug=False,
                   num_devices=NCORES)
    I = dict(
        biasM1=nc.dram_tensor("biasM1", [N_CHUNKS, S_TILES // 2, 128, 2 * CHUNK],
                              BF16, kind="ExternalInput").ap(),
        fsT9=nc.dram_tensor("fsT9", [S_TILES, 128, FS_PAD], F32R,
                            kind="ExternalInput").ap(),
        embsT=nc.dram_tensor("embsT", [EMB, N_SRC], F32R,
                             kind="ExternalInput").ap(),
        embdT=nc.dram_tensor("embdT", [EMB, M_PER], F32R,
                             kind="ExternalInput").ap(),
        attW=nc.dram_tensor("attW", [HEADS, EMB, HID], F32R,
                            kind="ExternalInput").ap(),
        attW2=nc.dram_tensor("attW2", [HEADS, HID, HID], F32R,
                             kind="ExternalInput").ap(),
        fcWT=nc.dram_tensor("fcWT", [KF, 128, TA_W], F32R,
                            kind="ExternalInput").ap(),
        decWT=nc.dram_tensor("decWT", [257, FEAT], F32R,
                             kind="ExternalInput").ap(),
        ident=nc.dram_tensor("ident", [128, 128], F32,
                             kind="ExternalInput").ap(),
        onesr=nc.dram_tensor("onesr", [1, 128], F32R,
                             kind="ExternalInput").ap(),
    )
    O = dict(
        out_re=nc.dram_tensor("out_re", [M_PER, TFD], F32,
                              kind="ExternalOutput").ap(),
        out_hat=nc.dram_tensor("out_hat", [M_PER, FEAT], F32,
                               kind="ExternalOutput").ap(),
    )
    with tile.TileContext(nc) as tc:
        _kernel_body(tc, I, O)
    return nc


def _get_program():
    global _PROGRAM
    if _PROGRAM is not None:
        return _PROGRAM
    nc = _build_program()
    _cap_pe_waits(nc)
    _PROGRAM = nc
    return nc


# ---------------- host side -------------------------------------------------
def _prep_in_maps(bias, emb_dest, emb_src, feature_src, fc_W, fc_b, dec_W,
                  dec_b, att_W, att_W2):
    f32 = np.float32
    # feature_src.T padded with a ones row (for fc_b) and zeros to 1152 rows,
    # rearranged so each src tile is one contiguous [128, 1152] DMA:
    # A[s, fi, k*128+si] = fsT_pad[k*128+fi, s*128+si]
    fsT = np.zeros((FS_PAD, N_SRC), f32)
    fsT[:FEAT] = feature_src.T
    fsT[FEAT] = 1.0
    A = np.ascontiguousarray(
        fsT.reshape(KF, 128, S_TILES, 128).transpose(2, 1, 0, 3)
    ).reshape(S_TILES, 128, FS_PAD)

    fcWT_a = np.zeros((FS_PAD, TA_W), f32)
    fcWT_a[:FEAT, :TFD] = fc_W.T
    fcWT_a[FEAT, :TFD] = fc_b
    fcWT_a[FEAT, 256] = 1.0
    fcWT9 = np.ascontiguousarray(fcWT_a.reshape(KF, 128, TA_W))

    decWT_a = np.zeros((257, FEAT), f32)
    decWT_a[:TFD] = dec_W.T
    decWT_a[256] = dec_b

    embsT_full = np.ascontiguousarray(emb_src.T)      # [64, 8192]
    biasT = bias.T                                    # [src, dest] view
    ident = np.eye(128, dtype=f32)
    onesr = np.ones((1, 128), f32)

    in_maps = []
    for c in range(NCORES):
        # roll src tiles so this core's own 8 tiles come first (uniform SPMD
        # program: feature_hat always uses tiles 0..7)
        order = (np.arange(S_TILES) + c * OWN_TILES) % S_TILES
        bs = biasT[:, c * M_PER:(c + 1) * M_PER]      # [8192, 1024]
        # mask M1: keep -> 1.0, masked -> -448 (exact in bf16); tiles laid out
        # [chunk][src-tile-pair][128 part][2*CHUNK] matching the paired
        # pipeline (halves = consecutive src tiles, same dest chunk)
        bt = bs.reshape(S_TILES, 128, N_CHUNKS, CHUNK)[order].transpose(2, 0, 1, 3)
        m1 = (bt.astype(f32) * 449.0 - 448.0).astype(ml_dtypes.bfloat16)
        m1 = np.ascontiguousarray(
            m1.reshape(N_CHUNKS, S_TILES // 2, 2, 128, CHUNK)
            .transpose(0, 1, 3, 2, 4)
            .reshape(N_CHUNKS, S_TILES // 2, 128, 2 * CHUNK))
        emT = np.ascontiguousarray(
            embsT_full.reshape(EMB, S_TILES, 128)[:, order]).reshape(EMB, N_SRC)
        in_maps.append(dict(
            biasM1=m1,
            fsT9=np.ascontiguousarray(A[order]),
            embsT=emT,
            embdT=np.ascontiguousarray(emb_dest[c * M_PER:(c + 1) * M_PER].T),
            attW=np.ascontiguousarray(att_W, dtype=f32),
            attW2=np.ascontiguousarray(att_W2, dtype=f32),
            fcWT=fcWT9,
            decWT=decWT_a,
            ident=ident,
            onesr=onesr,
        ))
    return in_maps


LAST_RESULTS = None


def kernel(bias, emb_dest, emb_src, feature_src, fc_W, fc_b, dec_W, dec_b,
           att_W, att_W2):
    global LAST_RESULTS
    from concourse.bass_utils import run_bass_kernel_spmd

    args = [np.asarray(x, np.float32) for x in
            (bias, emb_dest, emb_src, feature_src, fc_W, fc_b, dec_W, dec_b,
             att_W, att_W2)]
    in_maps = _prep_in_maps(*args)
    nc = _get_program()
    res = run_bass_kernel_spmd(nc, in_maps, core_ids=list(range(NCORES)))
    LAST_RESULTS = res
    out_re = np.concatenate([r["out_re"] for r in res.results], axis=0)
    out_hat = np.concatenate([r["out_hat"] for r in res.results], axis=0)
    return out_re.astype(np.float32), out_hat.astype(np.float32)


# revision 24
# speedup vs baseline: 1.1062x; 1.1062x over previous
"""Trainium2 Bass kernel: GNN message passing (nn_BaseAC_22505628631094).

Computation (see reference):
  transformed = feature_src @ fc_W.T + fc_b                     [N_src, TFD]
  h_src/h_dst = per-head emb @ att_W                            [H, N, HID]
  scores      = elu((h_dst @ att_W2) @ h_src.T)                 [H, N_dest, N_src]
  attn        = softmax(where(bias>0, scores, -inf), axis=-1)
  out_re      = mean_h(attn @ transformed)                      [N_dest, TFD]
  out_hat     = transformed @ dec_W.T + dec_b                   [N_src, FEAT]

Sharding: N_dest rows split across 8 cores (attention / softmax rows are
independent); feature_hat is sharded by N_src rows.  Everything is computed
in a transposed "scoresT" layout [src, dest] so that the attention matrix
comes out of the softmax directly usable as matmul lhsT for aggregation —
no on-device transposes of the big matrices.

Softmax numerator exp(elu(x))*mask is computed exactly with stock ops:
    u = exp(x)                       (ScalarE)
    z = min(u, M1), M1 in {1, -448}  (VectorE; clamp + mask in one op)
    v = exp(z - 1)                   (ScalarE; masked -> exp(-449) == 0)
    e = max(u, 1) * v                (VectorE scalar_tensor_tensor)
For x>0: v = exp(0) = 1, e = u = exp(x) exactly; for x<=0: e = exp(u-1) =
exp(elu(x)) exactly; masked entries are exactly 0.
Row sums (softmax denominators) fall out of the aggregation matmul via an
extra ones-column appended to `transformed`.
"""

import sys

import numpy as np

sys.path.insert(0, "/opt/trn_rl_repo")

import ml_dtypes  # noqa: E402

import concourse.bass as bass  # noqa: E402
import concourse.mybir as mybir  # noqa: E402
import concourse.tile as tile  # noqa: E402
from contextlib import ExitStack  # noqa: E402

# ---------------- problem constants (hardcoded per harness contract) -------
N_DEST = 8192
N_SRC = 8192
FEAT, TFD, EMB, HID, HEADS = 1024, 256, 64, 128, 2
NCORES = 8
M_PER = N_DEST // NCORES          # 1024 dest rows per core
S_TILES = N_SRC // 128            # 64 src tiles
CHUNK = 512                       # dest columns per (head, chunk) pass
N_CHUNKS = M_PER // CHUNK         # 2
DT_PER = CHUNK // 128             # 4 dest sub-tiles per chunk
KF = FEAT // 128 + 1              # 8 feature K-tiles + 1 bias-row tile
FS_PAD = KF * 128                 # 1152
OWN_TILES = S_TILES // NCORES     # 8 src tiles owned per core (feature_hat)
TA_W = 258                        # transformed_aug width: TFD + ones-col + pad
                                  # (col 256 = softmax-denominator ones column;
                                  # col 257 = zero pad: fp32r matmul needs even N)

BIAS_DT = mybir.dt.float8e4
BIAS_NP = ml_dtypes.float8_e4m3

# q(u) = Q0 + u*(Q1 + Q2*u) ~ exp((u-1)/2) on [0, 1]  (minimax fit, so that
# min(q,1)^2 ~ exp(u-1) with ~1.3e-3 max rel err; exact branch for u>1)
Q0, Q1, Q2 = 0.60692452, 0.29545556, 0.09697007

F32 = mybir.dt.float32
F32R = mybir.dt.float32r
BF16 = mybir.dt.bfloat16
EXP = mybir.ActivationFunctionType.Exp
MULT = mybir.AluOpType.mult


# ---------------- custom DVE op: fused exp(elu(.)) * mask ------------------
_EXP_ELU_OP = None


def _register_exp_elu_mask():
    global _EXP_ELU_OP
    if _EXP_ELU_OP is not None:
        return _EXP_ELU_OP
    import concourse.dve_ops as dvo
    from concourse.dve_spec import (
        C0, C1, C2, One, Spec, Src0, Src1, lower, maxx, minn, sq,
    )
    from concourse.dve_uop import DveOpSpec

    name = "EXP_ELU_MASK_ANT"
    if name in dvo._SUB_OPCODE_FOR_NAME:
        _EXP_ELU_OP = next(op for op in dvo.OPS if op.name == name)
        return _EXP_ELU_OP

    def _ref(in0, in1, s0, s1, imm2):
        u = np.asarray(in0, np.float32)
        q = np.float32(s0) + u * (np.float32(s1) + np.float32(imm2) * u)
        p = np.minimum(q, np.float32(1.0)) ** 2
        g = np.maximum(u, p)
        return (g * np.asarray(in1, np.float32)).astype(np.float32)

    body = maxx(Src0, sq(minn(C0 + Src0 * (C1 + C2 * Src0), One))) * Src1
    spec = Spec(body=body, reference=_ref)
    row = max(dvo._SUB_OPCODE_FOR_NAME.values()) + 1
    assert row < 0x20, "custom DVE opcode rows exhausted"
    shas = {}
    for ver in ("v3", "v4"):
        s = DveOpSpec(name=name, opcode=row, uops=lower(spec, ver=ver),
                      rd1_en=True)
        shas[ver] = s.sha(ver)
    op = dvo.DveOp(name, spec, subdim=False, uops_sha=shas)
    dvo.OPS.append(op)
    dvo.CUSTOM_DVE_SPECS[name] = spec
    dvo._SUB_OPCODE_FOR_NAME[name] = row
    _EXP_ELU_OP = op
    return op


# ---------------- device program -------------------------------------------
def _kernel_body(tc, I, O):
    nc = tc.nc

    with ExitStack() as ctx:
        # ---- whole-kernel SBUF pools ----
        consts = ctx.enter_context(tc.tile_pool(name="consts", bufs=1))
        big = ctx.enter_context(tc.tile_pool(name="big", bufs=1))
        tfa_pool = ctx.enter_context(tc.tile_pool(name="tfa", bufs=S_TILES))

        # constants
        fcWT = consts.tile([128, KF * TA_W], BF16, name="fcWT")
        for k in range(KF):
            nc.sync.dma_start(fcWT[:, k * TA_W:(k + 1) * TA_W], I["fcWT"][k])
        attW = consts.tile([EMB, HEADS * HID], F32R, name="attW")
        for h in range(HEADS):
            nc.sync.dma_start(attW[:, h * HID:(h + 1) * HID], I["attW"][h])
        attW2 = consts.tile([HID, HEADS * HID], F32R, name="attW2")
        for h in range(HEADS):
            nc.sync.dma_start(attW2[:, h * HID:(h + 1) * HID], I["attW2"][h])
        embdT = consts.tile([EMB, M_PER], F32R, name="embdT")
        nc.sync.dma_start(embdT[:], I["embdT"][:])
        ident = consts.tile([128, 128], F32, name="ident")
        nc.sync.dma_start(ident[:], I["ident"][:])
        onesr = consts.tile([1, 128], F32R, name="onesr")
        nc.sync.dma_start(onesr[:], I["onesr"][:])
        decA = consts.tile([128, FEAT], F32R, name="decA")
        nc.sync.dma_start(decA[:], I["decWT"][0:128, :])
        decB = consts.tile([128, FEAT], F32R, name="decB")
        nc.sync.dma_start(decB[:], I["decWT"][128:256, :])
        decC = consts.tile([1, FEAT], F32R, name="decC")
        nc.sync.dma_start(decC[:], I["decWT"][256:257, :])
        neg1 = consts.tile([128, 1], F32, name="neg1")
        nc.gpsimd.memset(neg1[:], -1.0)

        # big persistent tiles
        hsrc = big.tile([HID, HEADS * N_SRC], F32R, name="hsrc")   # 64KB/part
        hd2 = big.tile([HID, HEADS * M_PER], F32R, name="hd2")
        hdT = big.tile([HID, M_PER], F32R, name="hdT")
        oacc = big.tile([128, 8 * TFD], F32, name="oacc")
        own_tf = big.tile([128, OWN_TILES * TA_W], F32, name="own_tf")

        # ---- Phase A: h_srcT / hd2T for both heads --------------------------
        with tc.tile_pool(name="embs_p", bufs=1) as embs_p, \
             tc.tile_pool(name="ps_hs", bufs=2, space="PSUM") as ps_hs, \
             tc.tile_pool(name="ps_hd", bufs=1, space="PSUM") as ps_hd:
            embsT = embs_p.tile([EMB, N_SRC], F32R, name="embsT")
            nc.sync.dma_start(embsT[:], I["embsT"][:])
            for h in range(HEADS):
                aW = attW[:, h * HID:(h + 1) * HID]
                aW2 = attW2[:, h * HID:(h + 1) * HID]
                # h_srcT[h] = (emb_src @ att_W[h]).T = att_W[h].T @ emb_src.T
                for n in range(N_SRC // 512):
                    ph = ps_hs.tile([128, 512], F32, tag="hs", name="ph")
                    nc.tensor.matmul(ph[:], aW, embsT[:, n * 512:(n + 1) * 512],
                                     start=True, stop=True)
                    nc.vector.tensor_copy(hsrc[:, h * N_SRC + n * 512: h * N_SRC + (n + 1) * 512],
                                          ph[:])
                # hdT = att_W[h].T @ emb_destT ; hd2T = att_W2[h].T @ hdT
                pd = ps_hd.tile([128, M_PER], F32, tag="hd", name="pd")
                for n in range(M_PER // 512):
                    nc.tensor.matmul(pd[:, n * 512:(n + 1) * 512], aW,
                                     embdT[:, n * 512:(n + 1) * 512],
                                     start=True, stop=True)
                nc.vector.tensor_copy(hdT[:], pd[:])
                pd2 = ps_hd.tile([128, M_PER], F32, tag="hd", name="pd2")
                for n in range(M_PER // 512):
                    nc.tensor.matmul(pd2[:, n * 512:(n + 1) * 512], aW2,
                                     hdT[:, n * 512:(n + 1) * 512],
                                     start=True, stop=True)
                nc.vector.tensor_copy(hd2[:, h * M_PER + 0: h * M_PER + M_PER], pd2[:])

        # ---- Phases B-E: transformed (streamed) + attention main loop -------
        tfa = []  # 64 bf16 tiles [128, 257]: transformed_aug, ones in col 256

        # ---- transformed pre-phase (streams feature_src; overlaps phase A) --
        with tc.tile_pool(name="ps_tf", bufs=2, space="PSUM") as ps_tf, \
             tc.tile_pool(name="slab_p", bufs=3) as slab_p:
            for st in range(S_TILES):
                slab = slab_p.tile([128, FS_PAD], BF16, tag="slab", name="slab")
                nc.sync.dma_start(slab[:], I["fsT9"][st])
                pt = ps_tf.tile([128, TA_W], F32, tag="ptf", name="pt")
                for k in range(KF):
                    nc.tensor.matmul(pt[:], slab[:, k * 128:(k + 1) * 128],
                                     fcWT[:, k * TA_W:(k + 1) * TA_W],
                                     start=(k == 0), stop=(k == KF - 1))
                t = tfa_pool.tile([128, TA_W], BF16, tag="tfa", name="tfa_t")
                nc.scalar.copy(t[:], pt[:])
                if st < OWN_TILES:
                    nc.scalar.copy(own_tf[:, st * TA_W:(st + 1) * TA_W], pt[:])
                tfa.append(t)

        # ---- main attention loop: head x chunk x src-tile-pair --------------
        with tc.tile_pool(name="ps_sc", bufs=2, space="PSUM") as ps_sc, \
             tc.tile_pool(name="mask_p", bufs=6) as mask_p, \
             tc.tile_pool(name="ps_agg", bufs=4, space="PSUM") as ps_agg, \
             tc.tile_pool(name="u_p", bufs=4) as u_p, \
             tc.tile_pool(name="z_p", bufs=4) as z_p, \
             tc.tile_pool(name="v_p", bufs=4) as v_p, \
             tc.tile_pool(name="e_p", bufs=4) as e_p, \
             tc.tile_pool(name="nrm_p", bufs=2) as nrm_p:

            def main_iter(h, c, pr, agg):
                # two src tiles per iteration share one [128, 1024] pipeline
                m1 = mask_p.tile([128, 2 * CHUNK], BF16, tag="m1", name="m1")
                nc.sync.dma_start(m1[:], I["biasM1"][c, pr])
                ps = ps_sc.tile([128, 2 * CHUNK], F32, tag="sc", name="ps")
                for half in range(2):
                    st = 2 * pr + half
                    nc.tensor.matmul(
                        ps[:, half * CHUNK:(half + 1) * CHUNK],
                        hsrc[:, h * N_SRC + st * 128: h * N_SRC + (st + 1) * 128],
                        hd2[:, h * M_PER + c * CHUNK: h * M_PER + (c + 1) * CHUNK],
                        start=True, stop=True)
                u = u_p.tile([128, 2 * CHUNK], BF16, tag="u", name="u")
                nc.scalar.activation(u[:], ps[:], EXP)
                z = z_p.tile([128, 2 * CHUNK], BF16, tag="z", name="z")
                nc.vector.tensor_tensor(z[:], u[:], m1[:], mybir.AluOpType.min)
                v = v_p.tile([128, 2 * CHUNK], BF16, tag="v", name="v")
                nc.scalar.activation(v[:], z[:], EXP, bias=neg1[:])
                e = e_p.tile([128, 2 * CHUNK], BF16, tag="e", name="e")
                nc.vector.scalar_tensor_tensor(e[:], u[:], 1.0, v[:],
                                               mybir.AluOpType.max, MULT)
                for half in range(2):
                    st = 2 * pr + half
                    for dt in range(DT_PER):
                        nc.tensor.matmul(
                            agg[dt][:],
                            e[:, half * CHUNK + dt * 128: half * CHUNK + (dt + 1) * 128],
                            tfa[st][:], start=(st == 0),
                            stop=(st == S_TILES - 1))

            def normalize(h, c, agg):
                for dt in range(DT_PER):
                    rden = nrm_p.tile([128, 1], F32, tag="rden", name="rden")
                    nc.vector.reciprocal(rden[:], agg[dt][:, 256:257])
                    m = c * DT_PER + dt
                    dst = oacc[:, m * TFD:(m + 1) * TFD]
                    if h == 0:
                        nc.vector.tensor_scalar(dst, agg[dt][:, 0:256], rden[:],
                                                0.5, MULT, MULT)
                    else:
                        tmp = nrm_p.tile([128, TFD], F32, tag="ntmp", name="ntmp")
                        nc.vector.tensor_scalar(tmp[:], agg[dt][:, 0:256], rden[:],
                                                0.5, MULT, MULT)
                        nc.vector.tensor_add(dst, dst, tmp[:])

            pending = None
            for h in range(HEADS):
                for c in range(N_CHUNKS):
                    agg = [ps_agg.tile([128, TA_W], F32, tag="agg", name=f"agg{dt}")
                           for dt in range(DT_PER)]
                    for pr in range(S_TILES // 2):
                        main_iter(h, c, pr, agg)
                        if pr == 1 and pending is not None:
                            normalize(*pending)
                            pending = None
                    pending = (h, c, agg)
            normalize(*pending)

        # out_re DMA
        for m in range(8):
            nc.sync.dma_start(O["out_re"][m * 128:(m + 1) * 128, :],
                              oacc[:, m * TFD:(m + 1) * TFD])

        # ---- Phase F: feature_hat for this core's own 8 src tiles -----------
        with tc.tile_pool(name="ps_tr", bufs=2, space="PSUM") as ps_tr, \
             tc.tile_pool(name="ps_fh", bufs=2, space="PSUM") as ps_fh, \
             tc.tile_pool(name="hat_p", bufs=2) as hat_p:
            for j in range(OWN_TILES):
                tt = hat_p.tile([128, 256], F32R, tag="tt", name="tt")
                for kt in range(2):
                    ptr = ps_tr.tile([128, 128], F32, tag="tr", name="ptr")
                    nc.tensor.transpose(
                        ptr[:], own_tf[:, j * TA_W + kt * 128: j * TA_W + (kt + 1) * 128],
                        ident[:])
                    nc.scalar.copy(tt[:, kt * 128:(kt + 1) * 128], ptr[:])
                pfh = ps_fh.tile([128, FEAT], F32, tag="fh", name="pfh")
                for nf in range(2):
                    o = pfh[:, nf * 512:(nf + 1) * 512]
                    for kt in range(2):
                        nc.tensor.matmul(o, tt[:, kt * 128:(kt + 1) * 128],
                                         (decA if kt == 0 else decB)[:, nf * 512:(nf + 1) * 512],
                                         start=(kt == 0), stop=False)
                    nc.tensor.matmul(o, onesr[:], decC[:, nf * 512:(nf + 1) * 512],
                                     start=False, stop=True)
                fh = hat_p.tile([128, FEAT], F32, tag="fh_sb", name="fh")
                nc.vector.tensor_copy(fh[:], pfh[:])
                nc.sync.dma_start(O["out_hat"][j * 128:(j + 1) * 128, :], fh[:])


def _cap_pe_waits(nc):
    """Walrus codegen allows only one embedded sync-wait per compute-engine
    instruction (PE Matmult, ACT Activation, DVE ops, ...).  Tile's semaphore
    assignment can attach several; split the excess onto same-engine no-ops
    inserted immediately before — identical semantics, ~free."""
    import bass_rust
    k = 0
    # dummy semaphore for wait-carrier EVSEMs on SP (EVSEM needs an update)
    sem_names = dict(nc.m.ant_sem_names)
    dummy_id = max(int(i) for i in sem_names) + 1
    sem_names[str(dummy_id)] = ["wnop_dummy"]
    nc.m.ant_sem_names = sem_names
    dummy_upd = bass_rust.SyncUpdate(
        sync_type="semaphore", id=dummy_id, ant_name="wnop_dummy",
        update_mode="sem-inc", update_value=1, update_reg=None)
    skip = ("InstNoOp", "InstEventSemaphore",
            "InstAllEngineBarrier", "InstUnconditionalBranch", "InstISA",
            "InstBranchHint")
    cap_engines = {mybir.EngineType.PE, mybir.EngineType.DVE,
                   mybir.EngineType.Activation, mybir.EngineType.Pool,
                   mybir.EngineType.SP}
    for f in nc.m.functions:
        for blk in f.blocks:
            insts = blk.instructions
            out = []
            changed = False
            for inst in insts:
                if (type(inst).__name__ == "InstISA"
                        and getattr(inst, "op_name", None)
                        == "EVENT_SEMAPHORE_RANGE_CLEAR"):
                    # this walrus build rejects the encoding ("ISA wrong
                    # length"); the preceding reset-sema Drain already zeroes
                    # the semaphore range, so drop it
                    changed = True
                    continue
                si = inst.sync_info
                if (si is not None and type(inst).__name__ not in skip
                        and inst.engine in cap_engines):
                    waits = list(si.on_wait)
                    if len(waits) > 1:
                        for w in waits[:-1]:
                            if inst.engine == mybir.EngineType.SP:
                                nop = bass_rust.InstEventSemaphore(
                                    name=f"I-wnop{k}", ins=[], outs=[])
                                upd = [dummy_upd]
                            else:
                                nop = bass_rust.InstNoOp(
                                    name=f"I-wnop{k}", ins=[], outs=[])
                                upd = []
                            k += 1
                            nop.engine = inst.engine
                            nop.sync_info = bass_rust.SyncInfo(
                                on_wait=[w], on_update=upd)
                            out.append(nop)
                        si.on_wait = waits[-1:]
                        changed = True
                out.append(inst)
            if changed:
                blk.instructions = out
    return k


_PROGRAM = None


def _build_program():
    nc = bass.Bass("TRN2", target_bir_lowering=False, debug=False,
                   num_devices=NCORES)
    I = dict(
        biasM1=nc.dram_tensor("biasM1", [N_CHUNKS, S_TILES // 2, 128, 2 * CHUNK],
                              BF16, kind="ExternalInput").ap(),
        fsT9=nc.dram_tensor("fsT9", [S_TILES, 128, FS_PAD], BF16,
                            kind="ExternalInput").ap(),
        embsT=nc.dram_tensor("embsT", [EMB, N_SRC], F32R,
                             kind="ExternalInput").ap(),
        embdT=nc.dram_tensor("embdT", [EMB, M_PER], F32R,
                             kind="ExternalInput").ap(),
        attW=nc.dram_tensor("attW", [HEADS, EMB, HID], F32R,
                            kind="ExternalInput").ap(),
        attW2=nc.dram_tensor("attW2", [HEADS, HID, HID], F32R,
                             kind="ExternalInput").ap(),
        fcWT=nc.dram_tensor("fcWT", [KF, 128, TA_W], BF16,
                            kind="ExternalInput").ap(),
        decWT=nc.dram_tensor("decWT", [257, FEAT], F32R,
                             kind="ExternalInput").ap(),
        ident=nc.dram_tensor("ident", [128, 128], F32,
                             kind="ExternalInput").ap(),
        onesr=nc.dram_tensor("onesr", [1, 128], F32R,
                             kind="ExternalInput").ap(),
    )
    O = dict(
        out_re=nc.dram_tensor("out_re", [M_PER, TFD], F32,
                              kind="ExternalOutput").ap(),
        out_hat=nc.dram_tensor("out_hat", [M_PER, FEAT], F32,
                               kind="ExternalOutput").ap(),
    )
    with tile.TileContext(nc) as tc:
        _kernel_body(tc, I, O)
    return nc


def _get_program():
    global _PROGRAM
    if _PROGRAM is not None:
        return _PROGRAM
    nc = _build_program()
    _cap_pe_waits(nc)
    _PROGRAM = nc
    return nc


# ---------------- host side -------------------------------------------------
def _prep_in_maps(bias, emb_dest, emb_src, feature_src, fc_W, fc_b, dec_W,
                  dec_b, att_W, att_W2):
    f32 = np.float32
    # feature_src.T padded with a ones row (for fc_b) and zeros to 1152 rows,
    # rearranged so each src tile is one contiguous [128, 1152] DMA:
    # A[s, fi, k*128+si] = fsT_pad[k*128+fi, s*128+si]
    fsT = np.zeros((FS_PAD, N_SRC), f32)
    fsT[:FEAT] = feature_src.T
    fsT[FEAT] = 1.0
    A = np.ascontiguousarray(
        fsT.reshape(KF, 128, S_TILES, 128).transpose(2, 1, 0, 3)
    ).reshape(S_TILES, 128, FS_PAD).astype(ml_dtypes.bfloat16)

    fcWT_a = np.zeros((FS_PAD, TA_W), f32)
    fcWT_a[:FEAT, :TFD] = fc_W.T
    fcWT_a[FEAT, :TFD] = fc_b
    fcWT_a[FEAT, 256] = 1.0
    fcWT9 = np.ascontiguousarray(fcWT_a.reshape(KF, 128, TA_W)).astype(ml_dtypes.bfloat16)

    decWT_a = np.zeros((257, FEAT), f32)
    decWT_a[:TFD] = dec_W.T
    decWT_a[256] = dec_b

    embsT_full = np.ascontiguousarray(emb_src.T)      # [64, 8192]
    biasT = bias.T                                    # [src, dest] view
    ident = np.eye(128, dtype=f32)
    onesr = np.ones((1, 128), f32)

    in_maps = []
    for c in range(NCORES):
        # roll src tiles so this core's own 8 tiles come first (uniform SPMD
        # program: feature_hat always uses tiles 0..7)
        order = (np.arange(S_TILES) + c * OWN_TILES) % S_TILES
        bs = biasT[:, c * M_PER:(c + 1) * M_PER]      # [8192, 1024]
        # mask M1: keep -> 1.0, masked -> -448 (exact in bf16); tiles laid out
        # [chunk][src-tile-pair][128 part][2*CHUNK] matching the paired
        # pipeline (halves = consecutive src tiles, same dest chunk)
        bt = bs.reshape(S_TILES, 128, N_CHUNKS, CHUNK)[order].transpose(2, 0, 1, 3)
        m1 = (bt.astype(f32) * 449.0 - 448.0).astype(ml_dtypes.bfloat16)
        m1 = np.ascontiguousarray(
            m1.reshape(N_CHUNKS, S_TILES // 2, 2, 128, CHUNK)
            .transpose(0, 1, 3, 2, 4)
            .reshape(N_CHUNKS, S_TILES // 2, 128, 2 * CHUNK))
        emT = np.ascontiguousarray(
            embsT_full.reshape(EMB, S_TILES, 128)[:, order]).reshape(EMB, N_SRC)
        in_maps.append(dict(
            biasM1=m1,
            fsT9=np.ascontiguousarray(A[order]),
            embsT=emT,
            embdT=np.ascontiguousarray(emb_dest[c * M_PER:(c + 1) * M_PER].T),
            attW=np.ascontiguousarray(att_W, dtype=f32),
            attW2=np.ascontiguousarray(att_W2, dtype=f32),
            fcWT=fcWT9,
            decWT=decWT_a,
            ident=ident,
            onesr=onesr,
        ))
    return in_maps


LAST_RESULTS = None


def kernel(bias, emb_dest, emb_src, feature_src, fc_W, fc_b, dec_W, dec_b,
           att_W, att_W2):
    global LAST_RESULTS
    from concourse.bass_utils import run_bass_kernel_spmd

    args = [np.asarray(x, np.float32) for x in
            (bias, emb_dest, emb_src, feature_src, fc_W, fc_b, dec_W, dec_b,
             att_W, att_W2)]
    in_maps = _prep_in_maps(*args)
    nc = _get_program()
    res = run_bass_kernel_spmd(nc, in_maps, core_ids=list(range(NCORES)))
    LAST_RESULTS = res
    out_re = np.concatenate([r["out_re"] for r in res.results], axis=0)
    out_hat = np.concatenate([r["out_hat"] for r in res.results], axis=0)
    return out_re.astype(np.float32), out_hat.astype(np.float32)


# revision 25
# speedup vs baseline: 1.2703x; 1.1484x over previous
"""Trainium2 Bass kernel: GNN message passing (nn_BaseAC_22505628631094).

Computation (see reference):
  transformed = feature_src @ fc_W.T + fc_b                     [N_src, TFD]
  h_src/h_dst = per-head emb @ att_W                            [H, N, HID]
  scores      = elu((h_dst @ att_W2) @ h_src.T)                 [H, N_dest, N_src]
  attn        = softmax(where(bias>0, scores, -inf), axis=-1)
  out_re      = mean_h(attn @ transformed)                      [N_dest, TFD]
  out_hat     = transformed @ dec_W.T + dec_b                   [N_src, FEAT]

Sharding: N_dest rows split across 8 cores (attention / softmax rows are
independent); feature_hat is sharded by N_src rows.  Everything is computed
in a transposed "scoresT" layout [src, dest] so that the attention matrix
comes out of the softmax directly usable as matmul lhsT for aggregation —
no on-device transposes of the big matrices.

Softmax numerator exp(elu(x))*mask is computed exactly with stock ops:
    u = exp(x)                       (ScalarE)
    z = min(u, M1), M1 in {1, -448}  (VectorE; clamp + mask in one op)
    v = exp(z - 1)                   (ScalarE; masked -> exp(-449) == 0)
    e = max(u, 1) * v                (VectorE scalar_tensor_tensor)
For x>0: v = exp(0) = 1, e = u = exp(x) exactly; for x<=0: e = exp(u-1) =
exp(elu(x)) exactly; masked entries are exactly 0.
Row sums (softmax denominators) fall out of the aggregation matmul via an
extra ones-column appended to `transformed`.
"""

import sys

import numpy as np

sys.path.insert(0, "/opt/trn_rl_repo")

import ml_dtypes  # noqa: E402

import concourse.bass as bass  # noqa: E402
import concourse.mybir as mybir  # noqa: E402
import concourse.tile as tile  # noqa: E402
from contextlib import ExitStack  # noqa: E402

# ---------------- problem constants (hardcoded per harness contract) -------
N_DEST = 8192
N_SRC = 8192
FEAT, TFD, EMB, HID, HEADS = 1024, 256, 64, 128, 2
NCORES = 8
M_PER = N_DEST // NCORES          # 1024 dest rows per core
S_TILES = N_SRC // 128            # 64 src tiles
CHUNK = 512                       # dest columns per (head, chunk) pass
N_CHUNKS = M_PER // CHUNK         # 2
DT_PER = CHUNK // 128             # 4 dest sub-tiles per chunk
KF = FEAT // 128 + 1              # 8 feature K-tiles + 1 bias-row tile
FS_PAD = KF * 128                 # 1152
OWN_TILES = S_TILES // NCORES     # 8 src tiles owned per core (feature_hat)
TA_W = 258                        # transformed_aug width: TFD + ones-col + pad
                                  # (col 256 = softmax-denominator ones column;
                                  # col 257 = zero pad: fp32r matmul needs even N)

BIAS_DT = mybir.dt.float8e4
BIAS_NP = ml_dtypes.float8_e4m3

# q(u) = Q0 + u*(Q1 + Q2*u) ~ exp((u-1)/2) on [0, 1]  (minimax fit, so that
# min(q,1)^2 ~ exp(u-1) with ~1.3e-3 max rel err; exact branch for u>1)
Q0, Q1, Q2 = 0.60692452, 0.29545556, 0.09697007

F32 = mybir.dt.float32
F32R = mybir.dt.float32r
BF16 = mybir.dt.bfloat16
EXP = mybir.ActivationFunctionType.Exp
MULT = mybir.AluOpType.mult


# ---------------- custom DVE op: fused exp(elu(.)) * mask ------------------
_EXP_ELU_OP = None


def _register_exp_elu_mask():
    global _EXP_ELU_OP
    if _EXP_ELU_OP is not None:
        return _EXP_ELU_OP
    import concourse.dve_ops as dvo
    from concourse.dve_spec import (
        C0, C1, C2, One, Spec, Src0, Src1, lower, maxx, minn, sq,
    )
    from concourse.dve_uop import DveOpSpec

    name = "EXP_ELU_MASK_ANT"
    if name in dvo._SUB_OPCODE_FOR_NAME:
        _EXP_ELU_OP = next(op for op in dvo.OPS if op.name == name)
        return _EXP_ELU_OP

    def _ref(in0, in1, s0, s1, imm2):
        u = np.asarray(in0, np.float32)
        q = np.float32(s0) + u * (np.float32(s1) + np.float32(imm2) * u)
        p = np.minimum(q, np.float32(1.0)) ** 2
        g = np.maximum(u, p)
        return (g * np.asarray(in1, np.float32)).astype(np.float32)

    body = maxx(Src0, sq(minn(C0 + Src0 * (C1 + C2 * Src0), One))) * Src1
    spec = Spec(body=body, reference=_ref)
    row = max(dvo._SUB_OPCODE_FOR_NAME.values()) + 1
    assert row < 0x20, "custom DVE opcode rows exhausted"
    shas = {}
    for ver in ("v3", "v4"):
        s = DveOpSpec(name=name, opcode=row, uops=lower(spec, ver=ver),
                      rd1_en=True)
        shas[ver] = s.sha(ver)
    op = dvo.DveOp(name, spec, subdim=False, uops_sha=shas)
    dvo.OPS.append(op)
    dvo.CUSTOM_DVE_SPECS[name] = spec
    dvo._SUB_OPCODE_FOR_NAME[name] = row
    _EXP_ELU_OP = op
    return op


# ---------------- device program -------------------------------------------
def _kernel_body(tc, I, O):
    nc = tc.nc

    with ExitStack() as ctx:
        # ---- whole-kernel SBUF pools ----
        consts = ctx.enter_context(tc.tile_pool(name="consts", bufs=1))
        big = ctx.enter_context(tc.tile_pool(name="big", bufs=1))
        tfa_pool = ctx.enter_context(tc.tile_pool(name="tfa", bufs=S_TILES))

        # constants
        fcWT = consts.tile([128, KF * TA_W], BF16, name="fcWT")
        for k in range(KF):
            nc.sync.dma_start(fcWT[:, k * TA_W:(k + 1) * TA_W], I["fcWT"][k])
        attW = consts.tile([EMB, HEADS * HID], BF16, name="attW")
        for h in range(HEADS):
            nc.sync.dma_start(attW[:, h * HID:(h + 1) * HID], I["attW"][h])
        attW2 = consts.tile([HID, HEADS * HID], BF16, name="attW2")
        for h in range(HEADS):
            nc.sync.dma_start(attW2[:, h * HID:(h + 1) * HID], I["attW2"][h])
        embdT = consts.tile([EMB, M_PER], BF16, name="embdT")
        nc.sync.dma_start(embdT[:], I["embdT"][:])
        ident = consts.tile([128, 128], F32, name="ident")
        nc.sync.dma_start(ident[:], I["ident"][:])
        onesr = consts.tile([1, 128], F32R, name="onesr")
        nc.sync.dma_start(onesr[:], I["onesr"][:])
        decA = consts.tile([128, FEAT], F32R, name="decA")
        nc.sync.dma_start(decA[:], I["decWT"][0:128, :])
        decB = consts.tile([128, FEAT], F32R, name="decB")
        nc.sync.dma_start(decB[:], I["decWT"][128:256, :])
        decC = consts.tile([1, FEAT], F32R, name="decC")
        nc.sync.dma_start(decC[:], I["decWT"][256:257, :])
        neg1 = consts.tile([128, 1], F32, name="neg1")
        nc.gpsimd.memset(neg1[:], -1.0)

        # big persistent tiles
        hsrc = big.tile([HID, HEADS * N_SRC], BF16, name="hsrc")   # 64KB/part
        hd2 = big.tile([HID, HEADS * M_PER], BF16, name="hd2")
        hdT = big.tile([HID, M_PER], BF16, name="hdT")
        oacc = big.tile([128, 8 * TFD], F32, name="oacc")
        own_tf = big.tile([128, OWN_TILES * TA_W], F32, name="own_tf")

        # ---- Phase A: h_srcT / hd2T for both heads --------------------------
        with tc.tile_pool(name="embs_p", bufs=1) as embs_p, \
             tc.tile_pool(name="ps_hs", bufs=2, space="PSUM") as ps_hs, \
             tc.tile_pool(name="ps_hd", bufs=1, space="PSUM") as ps_hd:
            embsT = embs_p.tile([EMB, N_SRC], BF16, name="embsT")
            nc.sync.dma_start(embsT[:], I["embsT"][:])
            for h in range(HEADS):
                aW = attW[:, h * HID:(h + 1) * HID]
                aW2 = attW2[:, h * HID:(h + 1) * HID]
                # h_srcT[h] = (emb_src @ att_W[h]).T = att_W[h].T @ emb_src.T
                for n in range(N_SRC // 512):
                    ph = ps_hs.tile([128, 512], F32, tag="hs", name="ph")
                    nc.tensor.matmul(ph[:], aW, embsT[:, n * 512:(n + 1) * 512],
                                     start=True, stop=True)
                    nc.vector.tensor_copy(hsrc[:, h * N_SRC + n * 512: h * N_SRC + (n + 1) * 512],
                                          ph[:])
                # hdT = att_W[h].T @ emb_destT ; hd2T = att_W2[h].T @ hdT
                pd = ps_hd.tile([128, M_PER], F32, tag="hd", name="pd")
                for n in range(M_PER // 512):
                    nc.tensor.matmul(pd[:, n * 512:(n + 1) * 512], aW,
                                     embdT[:, n * 512:(n + 1) * 512],
                                     start=True, stop=True)
                nc.vector.tensor_copy(hdT[:], pd[:])
                pd2 = ps_hd.tile([128, M_PER], F32, tag="hd", name="pd2")
                for n in range(M_PER // 512):
                    nc.tensor.matmul(pd2[:, n * 512:(n + 1) * 512], aW2,
                                     hdT[:, n * 512:(n + 1) * 512],
                                     start=True, stop=True)
                nc.vector.tensor_copy(hd2[:, h * M_PER + 0: h * M_PER + M_PER], pd2[:])

        # ---- Phases B-E: transformed (streamed) + attention main loop -------
        tfa = []  # 64 bf16 tiles [128, 257]: transformed_aug, ones in col 256

        # ---- transformed pre-phase (streams feature_src; overlaps phase A) --
        with tc.tile_pool(name="ps_tf", bufs=2, space="PSUM") as ps_tf, \
             tc.tile_pool(name="slab_p", bufs=6) as slab_p:
            for st in range(S_TILES):
                slab = slab_p.tile([128, FS_PAD], BF16, tag="slab", name="slab")
                nc.sync.dma_start(slab[:], I["fsT9"][st])
                pt = ps_tf.tile([128, TA_W], F32, tag="ptf", name="pt")
                for k in range(KF):
                    nc.tensor.matmul(pt[:], slab[:, k * 128:(k + 1) * 128],
                                     fcWT[:, k * TA_W:(k + 1) * TA_W],
                                     start=(k == 0), stop=(k == KF - 1))
                t = tfa_pool.tile([128, TA_W], BF16, tag="tfa", name="tfa_t")
                nc.scalar.copy(t[:], pt[:])
                if st < OWN_TILES:
                    nc.scalar.copy(own_tf[:, st * TA_W:(st + 1) * TA_W], pt[:])
                tfa.append(t)

        # ---- main attention loop: head x chunk x src-tile-pair --------------
        with tc.tile_pool(name="ps_sc", bufs=2, space="PSUM") as ps_sc, \
             tc.tile_pool(name="mask_p", bufs=6) as mask_p, \
             tc.tile_pool(name="ps_agg", bufs=4, space="PSUM") as ps_agg, \
             tc.tile_pool(name="u_p", bufs=4) as u_p, \
             tc.tile_pool(name="z_p", bufs=4) as z_p, \
             tc.tile_pool(name="v_p", bufs=4) as v_p, \
             tc.tile_pool(name="e_p", bufs=4) as e_p, \
             tc.tile_pool(name="nrm_p", bufs=2) as nrm_p:

            def main_iter(h, c, pr, agg):
                # two src tiles per iteration share one [128, 1024] pipeline
                m1 = mask_p.tile([128, 2 * CHUNK], BF16, tag="m1", name="m1")
                nc.sync.dma_start(m1[:], I["biasM1"][c, pr])
                ps = ps_sc.tile([128, 2 * CHUNK], F32, tag="sc", name="ps")
                for half in range(2):
                    st = 2 * pr + half
                    nc.tensor.matmul(
                        ps[:, half * CHUNK:(half + 1) * CHUNK],
                        hsrc[:, h * N_SRC + st * 128: h * N_SRC + (st + 1) * 128],
                        hd2[:, h * M_PER + c * CHUNK: h * M_PER + (c + 1) * CHUNK],
                        start=True, stop=True)
                u = u_p.tile([128, 2 * CHUNK], BF16, tag="u", name="u")
                nc.scalar.activation(u[:], ps[:], EXP)
                z = z_p.tile([128, 2 * CHUNK], BF16, tag="z", name="z")
                nc.vector.tensor_tensor(z[:], u[:], m1[:], mybir.AluOpType.min)
                v = v_p.tile([128, 2 * CHUNK], BF16, tag="v", name="v")
                nc.scalar.activation(v[:], z[:], EXP, bias=neg1[:])
                e = e_p.tile([128, 2 * CHUNK], BF16, tag="e", name="e")
                nc.vector.scalar_tensor_tensor(e[:], u[:], 1.0, v[:],
                                               mybir.AluOpType.max, MULT)
                for half in range(2):
                    st = 2 * pr + half
                    for dt in range(DT_PER):
                        nc.tensor.matmul(
                            agg[dt][:],
                            e[:, half * CHUNK + dt * 128: half * CHUNK + (dt + 1) * 128],
                            tfa[st][:], start=(st == 0),
                            stop=(st == S_TILES - 1))

            def normalize(h, c, agg):
                for dt in range(DT_PER):
                    rden = nrm_p.tile([128, 1], F32, tag="rden", name="rden")
                    nc.vector.reciprocal(rden[:], agg[dt][:, 256:257])
                    m = c * DT_PER + dt
                    dst = oacc[:, m * TFD:(m + 1) * TFD]
                    if h == 0:
                        nc.vector.tensor_scalar(dst, agg[dt][:, 0:256], rden[:],
                                                0.5, MULT, MULT)
                    else:
                        tmp = nrm_p.tile([128, TFD], F32, tag="ntmp", name="ntmp")
                        nc.vector.tensor_scalar(tmp[:], agg[dt][:, 0:256], rden[:],
                                                0.5, MULT, MULT)
                        nc.vector.tensor_add(dst, dst, tmp[:])

            pending = None
            for h in range(HEADS):
                for c in range(N_CHUNKS):
                    agg = [ps_agg.tile([128, TA_W], F32, tag="agg", name=f"agg{dt}")
                           for dt in range(DT_PER)]
                    for pr in range(S_TILES // 2):
                        main_iter(h, c, pr, agg)
                        if pr == 1 and pending is not None:
                            normalize(*pending)
                            pending = None
                    pending = (h, c, agg)
            normalize(*pending)

        # out_re DMA
        for m in range(8):
            nc.sync.dma_start(O["out_re"][m * 128:(m + 1) * 128, :],
                              oacc[:, m * TFD:(m + 1) * TFD])

        # ---- Phase F: feature_hat for this core's own 8 src tiles -----------
        with tc.tile_pool(name="ps_tr", bufs=2, space="PSUM") as ps_tr, \
             tc.tile_pool(name="ps_fh", bufs=2, space="PSUM") as ps_fh, \
             tc.tile_pool(name="hat_p", bufs=2) as hat_p:
            for j in range(OWN_TILES):
                tt = hat_p.tile([128, 256], F32R, tag="tt", name="tt")
                for kt in range(2):
                    ptr = ps_tr.tile([128, 128], F32, tag="tr", name="ptr")
                    nc.tensor.transpose(
                        ptr[:], own_tf[:, j * TA_W + kt * 128: j * TA_W + (kt + 1) * 128],
                        ident[:])
                    nc.scalar.copy(tt[:, kt * 128:(kt + 1) * 128], ptr[:])
                pfh = ps_fh.tile([128, FEAT], F32, tag="fh", name="pfh")
                for nf in range(2):
                    o = pfh[:, nf * 512:(nf + 1) * 512]
                    for kt in range(2):
                        nc.tensor.matmul(o, tt[:, kt * 128:(kt + 1) * 128],
                                         (decA if kt == 0 else decB)[:, nf * 512:(nf + 1) * 512],
                                         start=(kt == 0), stop=False)
                    nc.tensor.matmul(o, onesr[:], decC[:, nf * 512:(nf + 1) * 512],
                                     start=False, stop=True)
                fh = hat_p.tile([128, FEAT], F32, tag="fh_sb", name="fh")
                nc.vector.tensor_copy(fh[:], pfh[:])
                nc.sync.dma_start(O["out_hat"][j * 128:(j + 1) * 128, :], fh[:])


def _cap_pe_waits(nc):
    """Walrus codegen allows only one embedded sync-wait per compute-engine
    instruction (PE Matmult, ACT Activation, DVE ops, ...).  Tile's semaphore
    assignment can attach several; split the excess onto same-engine no-ops
    inserted immediately before — identical semantics, ~free."""
    import bass_rust
    k = 0
    # dummy semaphore for wait-carrier EVSEMs on SP (EVSEM needs an update)
    sem_names = dict(nc.m.ant_sem_names)
    dummy_id = max(int(i) for i in sem_names) + 1
    sem_names[str(dummy_id)] = ["wnop_dummy"]
    nc.m.ant_sem_names = sem_names
    dummy_upd = bass_rust.SyncUpdate(
        sync_type="semaphore", id=dummy_id, ant_name="wnop_dummy",
        update_mode="sem-inc", update_value=1, update_reg=None)
    skip = ("InstNoOp", "InstEventSemaphore",
            "InstAllEngineBarrier", "InstUnconditionalBranch", "InstISA",
            "InstBranchHint")
    cap_engines = {mybir.EngineType.PE, mybir.EngineType.DVE,
                   mybir.EngineType.Activation, mybir.EngineType.Pool,
                   mybir.EngineType.SP}
    for f in nc.m.functions:
        for blk in f.blocks:
            insts = blk.instructions
            out = []
            changed = False
            for inst in insts:
                if (type(inst).__name__ == "InstISA"
                        and getattr(inst, "op_name", None)
                        == "EVENT_SEMAPHORE_RANGE_CLEAR"):
                    # this walrus build rejects the encoding ("ISA wrong
                    # length"); the preceding reset-sema Drain already zeroes
                    # the semaphore range, so drop it
                    changed = True
                    continue
                si = inst.sync_info
                if (si is not None and type(inst).__name__ not in skip
                        and inst.engine in cap_engines):
                    waits = list(si.on_wait)
                    if len(waits) > 1:
                        for w in waits[:-1]:
                            if inst.engine == mybir.EngineType.SP:
                                nop = bass_rust.InstEventSemaphore(
                                    name=f"I-wnop{k}", ins=[], outs=[])
                                upd = [dummy_upd]
                            else:
                                nop = bass_rust.InstNoOp(
                                    name=f"I-wnop{k}", ins=[], outs=[])
                                upd = []
                            k += 1
                            nop.engine = inst.engine
                            nop.sync_info = bass_rust.SyncInfo(
                                on_wait=[w], on_update=upd)
                            out.append(nop)
                        si.on_wait = waits[-1:]
                        changed = True
                out.append(inst)
            if changed:
                blk.instructions = out
    return k


_PROGRAM = None


def _build_program():
    nc = bass.Bass("TRN2", target_bir_lowering=False, debug=False,
                   num_devices=NCORES)
    I = dict(
        biasM1=nc.dram_tensor("biasM1", [N_CHUNKS, S_TILES // 2, 128, 2 * CHUNK],
                              BF16, kind="ExternalInput").ap(),
        fsT9=nc.dram_tensor("fsT9", [S_TILES, 128, FS_PAD], BF16,
                            kind="ExternalInput").ap(),
        embsT=nc.dram_tensor("embsT", [EMB, N_SRC], BF16,
                             kind="ExternalInput").ap(),
        embdT=nc.dram_tensor("embdT", [EMB, M_PER], BF16,
                             kind="ExternalInput").ap(),
        attW=nc.dram_tensor("attW", [HEADS, EMB, HID], BF16,
                            kind="ExternalInput").ap(),
        attW2=nc.dram_tensor("attW2", [HEADS, HID, HID], BF16,
                             kind="ExternalInput").ap(),
        fcWT=nc.dram_tensor("fcWT", [KF, 128, TA_W], BF16,
                            kind="ExternalInput").ap(),
        decWT=nc.dram_tensor("decWT", [257, FEAT], F32R,
                             kind="ExternalInput").ap(),
        ident=nc.dram_tensor("ident", [128, 128], F32,
                             kind="ExternalInput").ap(),
        onesr=nc.dram_tensor("onesr", [1, 128], F32R,
                             kind="ExternalInput").ap(),
    )
    O = dict(
        out_re=nc.dram_tensor("out_re", [M_PER, TFD], F32,
                              kind="ExternalOutput").ap(),
        out_hat=nc.dram_tensor("out_hat", [M_PER, FEAT], F32,
                               kind="ExternalOutput").ap(),
    )
    with tile.TileContext(nc) as tc:
        _kernel_body(tc, I, O)
    return nc


def _get_program():
    global _PROGRAM
    if _PROGRAM is not None:
        return _PROGRAM
    nc = _build_program()
    _cap_pe_waits(nc)
    _PROGRAM = nc
    return nc


# ---------------- host side -------------------------------------------------
def _prep_in_maps(bias, emb_dest, emb_src, feature_src, fc_W, fc_b, dec_W,
                  dec_b, att_W, att_W2):
    f32 = np.float32
    # feature_src.T padded with a ones row (for fc_b) and zeros to 1152 rows,
    # rearranged so each src tile is one contiguous [128, 1152] DMA:
    # A[s, fi, k*128+si] = fsT_pad[k*128+fi, s*128+si]
    fsT = np.zeros((FS_PAD, N_SRC), f32)
    fsT[:FEAT] = feature_src.T
    fsT[FEAT] = 1.0
    A = np.ascontiguousarray(
        fsT.reshape(KF, 128, S_TILES, 128).transpose(2, 1, 0, 3)
    ).reshape(S_TILES, 128, FS_PAD).astype(ml_dtypes.bfloat16)

    fcWT_a = np.zeros((FS_PAD, TA_W), f32)
    fcWT_a[:FEAT, :TFD] = fc_W.T
    fcWT_a[FEAT, :TFD] = fc_b
    fcWT_a[FEAT, 256] = 1.0
    fcWT9 = np.ascontiguousarray(fcWT_a.reshape(KF, 128, TA_W)).astype(ml_dtypes.bfloat16)

    decWT_a = np.zeros((257, FEAT), f32)
    decWT_a[:TFD] = dec_W.T
    decWT_a[256] = dec_b

    embsT_full = np.ascontiguousarray(emb_src.T)      # [64, 8192]
    biasT = bias.T                                    # [src, dest] view
    ident = np.eye(128, dtype=f32)
    onesr = np.ones((1, 128), f32)

    in_maps = []
    for c in range(NCORES):
        # roll src tiles so this core's own 8 tiles come first (uniform SPMD
        # program: feature_hat always uses tiles 0..7)
        order = (np.arange(S_TILES) + c * OWN_TILES) % S_TILES
        bs = biasT[:, c * M_PER:(c + 1) * M_PER]      # [8192, 1024]
        # mask M1: keep -> 1.0, masked -> -448 (exact in bf16); tiles laid out
        # [chunk][src-tile-pair][128 part][2*CHUNK] matching the paired
        # pipeline (halves = consecutive src tiles, same dest chunk)
        bt = bs.reshape(S_TILES, 128, N_CHUNKS, CHUNK)[order].transpose(2, 0, 1, 3)
        m1 = (bt.astype(f32) * 449.0 - 448.0).astype(ml_dtypes.bfloat16)
        m1 = np.ascontiguousarray(
            m1.reshape(N_CHUNKS, S_TILES // 2, 2, 128, CHUNK)
            .transpose(0, 1, 3, 2, 4)
            .reshape(N_CHUNKS, S_TILES // 2, 128, 2 * CHUNK))
        emT = np.ascontiguousarray(
            embsT_full.reshape(EMB, S_TILES, 128)[:, order]).reshape(EMB, N_SRC)
        in_maps.append(dict(
            biasM1=m1,
            fsT9=np.ascontiguousarray(A[order]),
            embsT=emT.astype(ml_dtypes.bfloat16),
            embdT=np.ascontiguousarray(emb_dest[c * M_PER:(c + 1) * M_PER].T).astype(ml_dtypes.bfloat16),
            attW=np.ascontiguousarray(att_W, dtype=f32).astype(ml_dtypes.bfloat16),
            attW2=np.ascontiguousarray(att_W2, dtype=f32).astype(ml_dtypes.bfloat16),
            fcWT=fcWT9,
            decWT=decWT_a,
            ident=ident,
            onesr=onesr,
        ))
    return in_maps


LAST_RESULTS = None


def kernel(bias, emb_dest, emb_src, feature_src, fc_W, fc_b, dec_W, dec_b,
           att_W, att_W2):
    global LAST_RESULTS
    from concourse.bass_utils import run_bass_kernel_spmd

    args = [np.asarray(x, np.float32) for x in
            (bias, emb_dest, emb_src, feature_src, fc_W, fc_b, dec_W, dec_b,
             att_W, att_W2)]
    in_maps = _prep_in_maps(*args)
    nc = _get_program()
    res = run_bass_kernel_spmd(nc, in_maps, core_ids=list(range(NCORES)))
    LAST_RESULTS = res
    out_re = np.concatenate([r["out_re"] for r in res.results], axis=0)
    out_hat = np.concatenate([r["out_hat"] for r in res.results], axis=0)
    return out_re.astype(np.float32), out_hat.astype(np.float32)
